# revision 13
# baseline (speedup 1.0000x reference)
"""Trainium2 Bass kernel for nn_MultiHeadAttention (B=4, T=2048, D=2048, H=16).

Sharding: tensor-parallel over heads. Each of 8 NeuronCores owns 2 heads
(256 of the 2048 Q/K/V dims). Matmuls run as float32r (measured ~230ns
sustained at N=512 across 8 cores, vs ~260ns for bf16 whose FWL weight
path is slower in-kernel).

Phase A (projections, all tokens): qT per head in [head_dim, tokens]
layout to a DRAM scratch, kT/v to HBM (kernel outputs). Weight loads are
staged (first Wq chunk -> first x tile -> rest; Wk/Wv on the ACT hwdge
queue) so the PE starts ~8us in.

Phase B (attention): scoresT[ktok, qtok] per 128-ktok block, three
blocks per PSUM group so exp runs as one [128,<=1536] ACT instruction
(amortizes the ~352-cycle ACT fixed cost and keeps ACT ~15% under the
PE rate); causal handling = block skip + triangle-mask multiply on the
diagonal 128x128 + column-sliced score/AV/den matmuls on diagonal
blocks. Softmax denominator via ones-matmul PSUM accumulation;
reciprocal_approx_fast. Normalized ctx for all batches stays in SBUF.

Phase C (output projection): MM-bound block stream, PSUM->SBUF copies
alternating DVE/ACT, out_p written bf16 (pure output, halves write DMA).

Host: Wo partials summed across cores in fp32; k/v slices concatenated.
"""

import os
import sys

import numpy as np

for _p in ("/opt/trn_rl_repo",):
    if _p not in sys.path and os.path.isdir(_p):
        sys.path.insert(0, _p)

B, T, D, H = 4, 2048, 2048, 16
HD = 128
N_CORES = 8
HPC = H // N_CORES          # heads per core
DPC = HPC * HD              # q/k/v dims per core
NTOK = B * T

P = 128
QT = 512                    # q-tile width
KC = 128                    # k-block
PT = 512                    # phase-A token tile
DSUB = 4                    # d-chunks per streamed xT tile
DIAG = QT // KC             # diagonal blocks per q-tile
G = 3                       # max k-blocks per exp group

_CACHE = {}


def _build_module():
    import concourse.bass as bass  # noqa: F401
    import concourse.mybir as mybir
    from concourse import bacc
    import concourse.tile as tile

    F32 = mybir.dt.float32
    F32R = mybir.dt.float32r
    BF16 = mybir.dt.bfloat16
    AF = mybir.ActivationFunctionType

    DK = D // P                 # 16 d-chunks
    NPT = NTOK // PT            # 16 phase-A tiles
    NQT = T // QT               # 4 q-tiles per batch
    TPB = T // P                # 16 token blocks per batch
    NOD = D // QT               # 4 output column chunks
    SCALE = 1.0 / float(np.sqrt(HD))

    nc = bacc.Bacc("TRN2", target_bir_lowering=False, debug=False)

    xT = nc.dram_tensor("xT", [D, NTOK], F32, kind="ExternalInput").ap()
    wqT = nc.dram_tensor("wqT", [D, DPC], F32, kind="ExternalInput").ap()
    wkT = nc.dram_tensor("wkT", [D, DPC], F32, kind="ExternalInput").ap()
    wvT = nc.dram_tensor("wvT", [D, DPC], F32, kind="ExternalInput").ap()
    woT = nc.dram_tensor("woT", [DPC, D], F32, kind="ExternalInput").ap()
    tri = nc.dram_tensor("tri", [KC, KC], F32, kind="ExternalInput").ap()

    kT_out = nc.dram_tensor("kT_out", [DPC, NTOK], F32, kind="ExternalOutput").ap()
    v_out = nc.dram_tensor("v_out", [NTOK, DPC], F32, kind="ExternalOutput").ap()
    out_p = nc.dram_tensor("out_p", [NTOK, D], BF16, kind="ExternalOutput").ap()

    xT_v = xT.rearrange("(dk p) t -> p dk t", p=P)
    wqT_v = wqT.rearrange("(dk p) n -> p dk n", p=P)
    wkT_v = wkT.rearrange("(dk p) n -> p dk n", p=P)
    wvT_v = wvT.rearrange("(dk p) n -> p dk n", p=P)
    woT_v = woT.rearrange("(hc p) n -> p hc n", p=P)
    v_out_v = v_out.rearrange("(c p) n -> p c n", p=P)

    def exp_groups(nkc):
        """Split nkc blocks into groups of at most G."""
        out = []
        kc = 0
        while kc < nkc:
            n = min(G, nkc - kc)
            out.append((kc, n))
            kc += n
        return out

    with tile.TileContext(nc) as tc:
        with tc.tile_pool(name="dram", bufs=1, space="DRAM") as dpool:
            q_scr = dpool.tile([HPC, P, NTOK], F32)

            # ---------------- Phase A: projections ----------------
            with (
                tc.tile_pool(name="wq", bufs=1) as wq_pool,
                tc.tile_pool(name="xt", bufs=2 * (DK // DSUB)) as xt_pool,
                tc.tile_pool(name="stA", bufs=4) as stA_pool,
                tc.tile_pool(name="pp_qk", bufs=2, space="PSUM") as pp_qk,
                tc.tile_pool(name="pp_v", bufs=2, space="PSUM") as pp_v,
            ):
                def load_xt(tb):
                    ts = slice(tb * PT, (tb + 1) * PT)
                    xts = []
                    for dg in range(DK // DSUB):
                        xt_t = xt_pool.tile([P, DSUB, PT], F32R, tag="xt")
                        nc.sync.dma_start(
                            xt_t[:],
                            xT_v[:, dg * DSUB:(dg + 1) * DSUB, ts].bitcast(F32R))
                        xts.append(xt_t)
                    return xts

                # stage the loads so the first q-matmul starts ~8us in:
                # Wq head chunk + first x tile on SP; Wk/Wv on the ACT
                # hwdge queue (idle during early phase A)
                wq_sb = wq_pool.tile([P, DK, DPC], F32R, tag="wq")
                wk_sb = wq_pool.tile([P, DK, DPC], F32R, tag="wk")
                wv_sb = wq_pool.tile([P, DK, DPC], F32R, tag="wv")
                nc.sync.dma_start(
                    wq_sb[:, 0:4, :], wqT_v[:, 0:4, :].bitcast(F32R))
                xts0 = load_xt(0)
                nc.sync.dma_start(
                    wq_sb[:, 4:DK, :], wqT_v[:, 4:DK, :].bitcast(F32R))
                nc.scalar.dma_start(wk_sb[:], wkT_v.bitcast(F32R))
                nc.scalar.dma_start(wv_sb[:], wvT_v.bitcast(F32R))

                for tb in range(NPT):
                    xts = xts0 if tb == 0 else load_xt(tb)
                    ts = slice(tb * PT, (tb + 1) * PT)

                    def xchunk(dc):
                        return xts[dc // DSUB][:, dc % DSUB, :]

                    for w_sb, is_q in ((wq_sb, True), (wk_sb, False)):
                        for hc in range(HPC):
                            ps = pp_qk.tile([P, PT], F32, tag="pqk")
                            for dc in range(DK):
                                nc.tensor.matmul(
                                    ps[:],
                                    w_sb[:, dc, hc * P:(hc + 1) * P],
                                    xchunk(dc),
                                    start=(dc == 0), stop=(dc == DK - 1))
                            st = stA_pool.tile([P, PT], F32, tag="stqk")
                            if is_q:
                                nc.vector.tensor_copy(st[:], ps[:])
                                nc.sync.dma_start(q_scr[hc, :, ts], st[:])
                            else:
                                nc.scalar.copy(st[:], ps[:])
                                nc.sync.dma_start(
                                    kT_out[hc * P:(hc + 1) * P, ts], st[:])

                    for sub in range(PT // P):
                        t0 = tb * PT + sub * P
                        ps = pp_v.tile([P, DPC], F32, tag="pv")
                        for dc in range(DK):
                            nc.tensor.matmul(
                                ps[:],
                                xchunk(dc)[:, sub * P:(sub + 1) * P],
                                wv_sb[:, dc, :],
                                start=(dc == 0), stop=(dc == DK - 1))
                        st = stA_pool.tile([P, DPC], F32, tag="stv")
                        if sub % 2 == 0:
                            nc.scalar.copy(st[:], ps[:])
                        else:
                            nc.vector.tensor_copy(st[:], ps[:])
                        nc.sync.dma_start(v_out[t0:t0 + P, :], st[:])

            # -------- Phases B (attention) + C (out-proj) --------
            with (
                tc.tile_pool(name="perB", bufs=1) as perB_pool,
                tc.tile_pool(name="ctx", bufs=B) as ctx_pool,
                tc.tile_pool(name="pair", bufs=2) as pair_pool,
                tc.tile_pool(name="exp", bufs=3) as exp_pool,
                tc.tile_pool(name="rcp", bufs=2) as rcp_pool,
                tc.tile_pool(name="ost", bufs=3) as ost_pool,
            ):
                wo_sb = perB_pool.tile([P, HPC, D], F32R, tag="wo")
                mask_sb = perB_pool.tile([P, KC], F32R, tag="mask")
                ones_f = perB_pool.tile([P, P], F32, tag="onesf")
                ones_sb = perB_pool.tile([P, P], F32R, tag="ones")
                nc.sync.dma_start(mask_sb[:], tri.bitcast(F32R))
                nc.vector.memset(ones_f[:], 1.0)
                nc.vector.tensor_copy(ones_sb[:], ones_f[:])

                ctx_ts = []
                with (
                    tc.tile_pool(name="pp_s", bufs=2, space="PSUM") as pp_s,
                    tc.tile_pool(name="pp_ctx", bufs=1, space="PSUM") as pp_ctx,
                    tc.tile_pool(name="pp_den", bufs=1, space="PSUM") as pp_den,
                ):
                    # initialize the two score PSUM buffers once: sliced
                    # diagonal score matmuls leave stale columns that exp
                    # reads (their e values are never consumed downstream)
                    for _ in range(2):
                        s_init = pp_s.tile([P, G, QT], F32, tag="ps")
                        nc.vector.memset(s_init[:], 0.0)

                    def do_pair(b, h, ctx_t):
                        qt_pair = pair_pool.tile([P, T], F32R, tag="qpair")
                        kt_pair = pair_pool.tile([P, T], F32R, tag="kpair")
                        v_pair = pair_pool.tile([P, TPB, HD], F32R, tag="vpair")
                        bs = slice(b * T, (b + 1) * T)
                        nc.sync.dma_start(
                            qt_pair[:], q_scr[h, :, bs].bitcast(F32R))
                        nc.sync.dma_start(
                            kt_pair[:],
                            kT_out[h * P:(h + 1) * P, bs].bitcast(F32R))
                        nc.sync.dma_start(
                            v_pair[:],
                            v_out_v[:, b * TPB:(b + 1) * TPB,
                                    h * HD:(h + 1) * HD].bitcast(F32R))

                        for qt in range(NQT):
                            nkc = (qt + 1) * DIAG
                            ctx_ps = pp_ctx.tile([P, QT], F32, tag="pctx")
                            den_ps = pp_den.tile([P, QT], F32, tag="pden")
                            for kc0, gn in exp_groups(nkc):
                                s_ps = pp_s.tile([P, G, QT], F32, tag="ps")
                                for u in range(gn):
                                    kc = kc0 + u
                                    j = kc - qt * DIAG
                                    lo = j * KC if j >= 0 else 0
                                    nc.tensor.matmul(
                                        s_ps[:, u, lo:],
                                        kt_pair[:, kc * KC:(kc + 1) * KC],
                                        qt_pair[:, qt * QT + lo:(qt + 1) * QT],
                                        start=True, stop=True,
                                        skip_group_check=True)
                                e_g = exp_pool.tile([P, G, QT], F32R, tag="eg")
                                nc.scalar.activation(
                                    e_g[:, 0:gn, :], s_ps[:, 0:gn, :],
                                    AF.Exp, scale=SCALE)
                                for u in range(gn):
                                    kc = kc0 + u
                                    j = kc - qt * DIAG
                                    lo = j * KC if j >= 0 else 0
                                    if j >= 0:
                                        nc.vector.tensor_mul(
                                            e_g[:, u, j * KC:(j + 1) * KC],
                                            e_g[:, u, j * KC:(j + 1) * KC],
                                            mask_sb[:])
                                    nc.tensor.matmul(
                                        ctx_ps[:, lo:],
                                        v_pair[:, kc, :],
                                        e_g[:, u, lo:],
                                        start=(kc == 0), stop=(kc == nkc - 1),
                                        skip_group_check=True)
                                    nc.tensor.matmul(
                                        den_ps[:, lo:],
                                        ones_sb[:],
                                        e_g[:, u, lo:],
                                        start=(kc == 0), stop=(kc == nkc - 1),
                                        skip_group_check=True)
                            rcp = rcp_pool.tile([P, QT], F32, tag="rcp")
                            nc.vector.reciprocal_approx_fast(rcp[:], den_ps[:])
                            nc.vector.tensor_mul(
                                ctx_t[:, h, qt * QT:(qt + 1) * QT],
                                ctx_ps[:], rcp[:])

                    for b in range(B):
                        if b == 1:
                            # wo arrives during attention, ready for phase C
                            nc.sync.dma_start(wo_sb[:], woT_v.bitcast(F32R))
                        ctx_t = ctx_pool.tile([P, HPC, T], F32R, tag="ctx")
                        ctx_ts.append(ctx_t)
                        for h in range(HPC):
                            do_pair(b, h, ctx_t)

                # ---------------- Phase C: output projection ----------------
                with tc.tile_pool(name="pp_o", bufs=4, space="PSUM") as pp_o:
                    ncopy = 0
                    for b in range(B):
                        ctx_t = ctx_ts[b]
                        for tb2 in range(TPB):
                            t0 = b * T + tb2 * P
                            ost = ost_pool.tile([P, D], BF16, tag="ost")
                            for od in range(NOD):
                                ods = slice(od * QT, (od + 1) * QT)
                                ps0 = pp_o.tile([P, QT], F32, tag="po")
                                nc.tensor.matmul(
                                    ps0[:], ctx_t[:, 0, tb2 * P:(tb2 + 1) * P],
                                    wo_sb[:, 0, ods], start=True, stop=False)
                                nc.tensor.matmul(
                                    ps0[:], ctx_t[:, 1, tb2 * P:(tb2 + 1) * P],
                                    wo_sb[:, 1, ods], start=False, stop=True)
                                if ncopy % 2 == 0:
                                    nc.vector.tensor_copy(ost[:, ods], ps0[:])
                                else:
                                    nc.scalar.copy(ost[:, ods], ps0[:])
                                ncopy += 1
                            nc.sync.dma_start(out_p[t0:t0 + P, :], ost[:])

    nc.compile()
    return nc


def _get_module():
    if "nc" not in _CACHE:
        _CACHE["nc"] = _build_module()
    return _CACHE["nc"]


def _make_tri():
    return (np.arange(KC)[:, None] <= np.arange(KC)[None, :]).astype(np.float32)


def _run(x, Wq, Wk, Wv, Wo, bo, trace=False):
    from concourse import bass_utils

    nc = _get_module()
    x = np.asarray(x, dtype=np.float32)
    xT = np.ascontiguousarray(x.reshape(NTOK, D).T)
    tri = _make_tri()
    Wq = np.asarray(Wq, np.float32)
    Wk = np.asarray(Wk, np.float32)
    Wv = np.asarray(Wv, np.float32)
    Wo = np.asarray(Wo, np.float32)
    in_maps = []
    for c in range(N_CORES):
        sl = slice(c * DPC, (c + 1) * DPC)
        in_maps.append({
            "xT": xT,
            "wqT": np.ascontiguousarray(Wq[sl, :].T),
            "wkT": np.ascontiguousarray(Wk[sl, :].T),
            "wvT": np.ascontiguousarray(Wv[sl, :].T),
            "woT": np.ascontiguousarray(Wo[:, sl].T),
            "tri": tri,
        })
    res = bass_utils.run_bass_kernel_spmd(
        nc, in_maps, core_ids=list(range(N_CORES)), trace=trace)

    out = np.zeros((NTOK, D), np.float32)
    k = np.empty((NTOK, D), np.float32)
    v = np.empty((NTOK, D), np.float32)
    for c, r in enumerate(res.results):
        sl = slice(c * DPC, (c + 1) * DPC)
        out += np.asarray(r["out_p"]).astype(np.float32)
        k[:, sl] = np.asarray(r["kT_out"]).T
        v[:, sl] = np.asarray(r["v_out"])
    out += np.asarray(bo, np.float32)[None, :]
    outs = (out.reshape(B, T, D), k.reshape(B, T, D), v.reshape(B, T, D))
    return outs, res


def kernel(x, Wq, Wk, Wv, Wo, bo):
    outs, _ = _run(x, Wq, Wk, Wv, Wo, bo, trace=False)
    return outs


# revision 15
# speedup vs baseline: 1.0456x; 1.0456x over previous
"""Trainium2 Bass kernel for nn_MultiHeadAttention (B=4, T=2048, D=2048, H=16).

Sharding: tensor-parallel over heads. Each of 8 NeuronCores owns 2 heads
(256 of the 2048 Q/K/V dims). Matmuls run as float32r (measured ~230ns
sustained at N=512 across 8 cores, vs ~260ns for bf16 whose FWL weight
path is slower in-kernel).

Phase A (projections, all tokens): qT per head in [head_dim, tokens]
layout to a DRAM scratch, kT/v to HBM (kernel outputs). Weight loads are
staged (first Wq chunk -> first x tile -> rest; Wk/Wv on the ACT hwdge
queue) so the PE starts ~8us in.

Phase B (attention): scoresT[ktok, qtok] per 128-ktok block, three
blocks per PSUM group so exp runs as one [128,<=1536] ACT instruction
(amortizes the ~352-cycle ACT fixed cost and keeps ACT ~15% under the
PE rate); causal handling = block skip + triangle-mask multiply on the
diagonal 128x128 + column-sliced score/AV/den matmuls on diagonal
blocks. Softmax denominator via ones-matmul PSUM accumulation;
reciprocal_approx_fast. Normalized ctx for all batches stays in SBUF.

Phase C (output projection): MM-bound block stream, PSUM->SBUF copies
alternating DVE/ACT, out_p written bf16 (pure output, halves write DMA).

Host: Wo partials summed across cores in fp32; k/v slices concatenated.
"""

import os
import sys

import numpy as np

for _p in ("/opt/trn_rl_repo",):
    if _p not in sys.path and os.path.isdir(_p):
        sys.path.insert(0, _p)

B, T, D, H = 4, 2048, 2048, 16
HD = 128
N_CORES = 8
HPC = H // N_CORES          # heads per core
DPC = HPC * HD              # q/k/v dims per core
NTOK = B * T

P = 128
QT = 512                    # q-tile width
KC = 128                    # k-block
PT = 512                    # phase-A token tile
DSUB = 4                    # d-chunks per streamed xT tile
DIAG = QT // KC             # diagonal blocks per q-tile
G = 2                       # max k-blocks per exp group

_CACHE = {}


def _build_module():
    import concourse.bass as bass  # noqa: F401
    import concourse.mybir as mybir
    from concourse import bacc
    import concourse.tile as tile

    F32 = mybir.dt.float32
    F32R = mybir.dt.float32r
    BF16 = mybir.dt.bfloat16
    AF = mybir.ActivationFunctionType

    DK = D // P                 # 16 d-chunks
    NPT = NTOK // PT            # 16 phase-A tiles
    NQT = T // QT               # 4 q-tiles per batch
    TPB = T // P                # 16 token blocks per batch
    NOD = D // QT               # 4 output column chunks
    SCALE = 1.0 / float(np.sqrt(HD))

    nc = bacc.Bacc("TRN2", target_bir_lowering=False, debug=False)

    xT = nc.dram_tensor("xT", [D, NTOK], F32, kind="ExternalInput").ap()
    wqT = nc.dram_tensor("wqT", [D, DPC], F32, kind="ExternalInput").ap()
    wkT = nc.dram_tensor("wkT", [D, DPC], F32, kind="ExternalInput").ap()
    wvT = nc.dram_tensor("wvT", [D, DPC], F32, kind="ExternalInput").ap()
    woT = nc.dram_tensor("woT", [DPC, D], F32, kind="ExternalInput").ap()
    tri = nc.dram_tensor("tri", [KC, KC], F32, kind="ExternalInput").ap()

    kT_out = nc.dram_tensor("kT_out", [DPC, NTOK], F32, kind="ExternalOutput").ap()
    v_out = nc.dram_tensor("v_out", [NTOK, DPC], F32, kind="ExternalOutput").ap()
    out_p = nc.dram_tensor("out_p", [NTOK, D], BF16, kind="ExternalOutput").ap()

    xT_v = xT.rearrange("(dk p) t -> p dk t", p=P)
    wqT_v = wqT.rearrange("(dk p) n -> p dk n", p=P)
    wkT_v = wkT.rearrange("(dk p) n -> p dk n", p=P)
    wvT_v = wvT.rearrange("(dk p) n -> p dk n", p=P)
    woT_v = woT.rearrange("(hc p) n -> p hc n", p=P)
    v_out_v = v_out.rearrange("(c p) n -> p c n", p=P)

    def exp_groups(nkc):
        """Split nkc blocks into groups of at most G."""
        out = []
        kc = 0
        while kc < nkc:
            n = min(G, nkc - kc)
            out.append((kc, n))
            kc += n
        return out

    with tile.TileContext(nc) as tc:
        with tc.tile_pool(name="dram", bufs=1, space="DRAM") as dpool:
            q_scr = dpool.tile([HPC, P, NTOK], F32)

            # ---------------- Phase A: projections ----------------
            with (
                tc.tile_pool(name="wq", bufs=1) as wq_pool,
                tc.tile_pool(name="xt", bufs=2 * (DK // DSUB)) as xt_pool,
                tc.tile_pool(name="stA", bufs=4) as stA_pool,
                tc.tile_pool(name="pp_qk", bufs=2, space="PSUM") as pp_qk,
                tc.tile_pool(name="pp_v", bufs=2, space="PSUM") as pp_v,
            ):
                def load_xt(tb):
                    ts = slice(tb * PT, (tb + 1) * PT)
                    xts = []
                    for dg in range(DK // DSUB):
                        xt_t = xt_pool.tile([P, DSUB, PT], F32R, tag="xt")
                        nc.sync.dma_start(
                            xt_t[:],
                            xT_v[:, dg * DSUB:(dg + 1) * DSUB, ts].bitcast(F32R))
                        xts.append(xt_t)
                    return xts

                wq_sb = wq_pool.tile([P, DK, DPC], F32R, tag="wq")
                wk_sb = wq_pool.tile([P, DK, DPC], F32R, tag="wk")
                wv_sb = wq_pool.tile([P, DK, DPC], F32R, tag="wv")
                nc.sync.dma_start(wq_sb[:], wqT_v.bitcast(F32R))
                xts0 = load_xt(0)
                nc.sync.dma_start(wk_sb[:], wkT_v.bitcast(F32R))
                nc.sync.dma_start(wv_sb[:], wvT_v.bitcast(F32R))

                for tb in range(NPT):
                    xts = xts0 if tb == 0 else load_xt(tb)
                    ts = slice(tb * PT, (tb + 1) * PT)

                    def xchunk(dc):
                        return xts[dc // DSUB][:, dc % DSUB, :]

                    for w_sb, is_q in ((wq_sb, True), (wk_sb, False)):
                        for hc in range(HPC):
                            ps = pp_qk.tile([P, PT], F32, tag="pqk")
                            for dc in range(DK):
                                nc.tensor.matmul(
                                    ps[:],
                                    w_sb[:, dc, hc * P:(hc + 1) * P],
                                    xchunk(dc),
                                    start=(dc == 0), stop=(dc == DK - 1))
                            st = stA_pool.tile([P, PT], F32, tag="stqk")
                            if is_q:
                                nc.vector.tensor_copy(st[:], ps[:])
                                nc.sync.dma_start(q_scr[hc, :, ts], st[:])
                            else:
                                nc.scalar.copy(st[:], ps[:])
                                nc.sync.dma_start(
                                    kT_out[hc * P:(hc + 1) * P, ts], st[:])

                    for sub in range(PT // P):
                        t0 = tb * PT + sub * P
                        ps = pp_v.tile([P, DPC], F32, tag="pv")
                        for dc in range(DK):
                            nc.tensor.matmul(
                                ps[:],
                                xchunk(dc)[:, sub * P:(sub + 1) * P],
                                wv_sb[:, dc, :],
                                start=(dc == 0), stop=(dc == DK - 1))
                        st = stA_pool.tile([P, DPC], F32, tag="stv")
                        if sub % 2 == 0:
                            nc.scalar.copy(st[:], ps[:])
                        else:
                            nc.vector.tensor_copy(st[:], ps[:])
                        nc.sync.dma_start(v_out[t0:t0 + P, :], st[:])

            # -------- Phases B (attention) + C (out-proj) --------
            with (
                tc.tile_pool(name="perB", bufs=1) as perB_pool,
                tc.tile_pool(name="ctx", bufs=B) as ctx_pool,
                tc.tile_pool(name="pair", bufs=2) as pair_pool,
                tc.tile_pool(name="exp", bufs=3) as exp_pool,
                tc.tile_pool(name="rcp", bufs=2) as rcp_pool,
                tc.tile_pool(name="ost", bufs=3) as ost_pool,
            ):
                wo_sb = perB_pool.tile([P, HPC, D], F32R, tag="wo")
                mask_sb = perB_pool.tile([P, KC], F32R, tag="mask")
                ones_f = perB_pool.tile([P, P], F32, tag="onesf")
                ones_sb = perB_pool.tile([P, P], F32R, tag="ones")
                nc.sync.dma_start(mask_sb[:], tri.bitcast(F32R))
                nc.vector.memset(ones_f[:], 1.0)
                nc.vector.tensor_copy(ones_sb[:], ones_f[:])

                ctx_ts = []
                with (
                    tc.tile_pool(name="pp_s", bufs=2, space="PSUM") as pp_s,
                    tc.tile_pool(name="pp_ctx", bufs=2, space="PSUM") as pp_ctx,
                    tc.tile_pool(name="pp_den", bufs=2, space="PSUM") as pp_den,
                ):
                    # initialize the two score PSUM buffers once: sliced
                    # diagonal score matmuls leave stale columns that exp
                    # reads (their e values are never consumed downstream)
                    for _ in range(2):
                        s_init = pp_s.tile([P, G, QT], F32, tag="ps")
                        nc.vector.memset(s_init[:], 0.0)

                    def do_pair(b, h, ctx_t):
                        qt_pair = pair_pool.tile([P, T], F32R, tag="qpair")
                        kt_pair = pair_pool.tile([P, T], F32R, tag="kpair")
                        v_pair = pair_pool.tile([P, TPB, HD], F32R, tag="vpair")
                        bs = slice(b * T, (b + 1) * T)
                        nc.sync.dma_start(
                            qt_pair[:], q_scr[h, :, bs].bitcast(F32R))
                        nc.sync.dma_start(
                            kt_pair[:],
                            kT_out[h * P:(h + 1) * P, bs].bitcast(F32R))
                        nc.sync.dma_start(
                            v_pair[:],
                            v_out_v[:, b * TPB:(b + 1) * TPB,
                                    h * HD:(h + 1) * HD].bitcast(F32R))

                        for qt in range(NQT):
                            nkc = (qt + 1) * DIAG
                            ctx_ps = pp_ctx.tile([P, QT], F32, tag="pctx")
                            den_ps = pp_den.tile([P, QT], F32, tag="pden")
                            for kc0, gn in exp_groups(nkc):
                                s_ps = pp_s.tile([P, G, QT], F32, tag="ps")
                                for u in range(gn):
                                    kc = kc0 + u
                                    j = kc - qt * DIAG
                                    lo = j * KC if j >= 0 else 0
                                    nc.tensor.matmul(
                                        s_ps[:, u, lo:],
                                        kt_pair[:, kc * KC:(kc + 1) * KC],
                                        qt_pair[:, qt * QT + lo:(qt + 1) * QT],
                                        start=True, stop=True,
                                        skip_group_check=True)
                                e_g = exp_pool.tile([P, G, QT], F32R, tag="eg")
                                nc.scalar.activation(
                                    e_g[:, 0:gn, :], s_ps[:, 0:gn, :],
                                    AF.Exp, scale=SCALE)
                                for u in range(gn):
                                    kc = kc0 + u
                                    j = kc - qt * DIAG
                                    lo = j * KC if j >= 0 else 0
                                    if j >= 0:
                                        nc.vector.tensor_mul(
                                            e_g[:, u, j * KC:(j + 1) * KC],
                                            e_g[:, u, j * KC:(j + 1) * KC],
                                            mask_sb[:])
                                    nc.tensor.matmul(
                                        ctx_ps[:, lo:],
                                        v_pair[:, kc, :],
                                        e_g[:, u, lo:],
                                        start=(kc == 0), stop=(kc == nkc - 1),
                                        skip_group_check=True)
                                    nc.tensor.matmul(
                                        den_ps[:, lo:],
                                        ones_sb[:],
                                        e_g[:, u, lo:],
                                        start=(kc == 0), stop=(kc == nkc - 1),
                                        skip_group_check=True)
                            rcp = rcp_pool.tile([P, QT], F32, tag="rcp")
                            nc.vector.reciprocal_approx_fast(rcp[:], den_ps[:])
                            nc.vector.tensor_mul(
                                ctx_t[:, h, qt * QT:(qt + 1) * QT],
                                ctx_ps[:], rcp[:])

                    for b in range(B):
                        if b == 1:
                            # wo arrives during attention, ready for phase C
                            nc.sync.dma_start(wo_sb[:], woT_v.bitcast(F32R))
                        ctx_t = ctx_pool.tile([P, HPC, T], F32R, tag="ctx")
                        ctx_ts.append(ctx_t)
                        for h in range(HPC):
                            do_pair(b, h, ctx_t)

                # ---------------- Phase C: output projection ----------------
                with tc.tile_pool(name="pp_o", bufs=4, space="PSUM") as pp_o:
                    ncopy = 0
                    for b in range(B):
                        ctx_t = ctx_ts[b]
                        for tb2 in range(TPB):
                            t0 = b * T + tb2 * P
                            ost = ost_pool.tile([P, D], BF16, tag="ost")
                            for od in range(NOD):
                                ods = slice(od * QT, (od + 1) * QT)
                                ps0 = pp_o.tile([P, QT], F32, tag="po")
                                nc.tensor.matmul(
                                    ps0[:], ctx_t[:, 0, tb2 * P:(tb2 + 1) * P],
                                    wo_sb[:, 0, ods], start=True, stop=False)
                                nc.tensor.matmul(
                                    ps0[:], ctx_t[:, 1, tb2 * P:(tb2 + 1) * P],
                                    wo_sb[:, 1, ods], start=False, stop=True)
                                if ncopy % 2 == 0:
                                    nc.vector.tensor_copy(ost[:, ods], ps0[:])
                                else:
                                    nc.scalar.copy(ost[:, ods], ps0[:])
                                ncopy += 1
                            nc.sync.dma_start(out_p[t0:t0 + P, :], ost[:])

    nc.compile()
    return nc


def _get_module():
    if "nc" not in _CACHE:
        _CACHE["nc"] = _build_module()
    return _CACHE["nc"]


def _make_tri():
    return (np.arange(KC)[:, None] <= np.arange(KC)[None, :]).astype(np.float32)


def _run(x, Wq, Wk, Wv, Wo, bo, trace=False):
    from concourse import bass_utils

    nc = _get_module()
    x = np.asarray(x, dtype=np.float32)
    xT = np.ascontiguousarray(x.reshape(NTOK, D).T)
    tri = _make_tri()
    Wq = np.asarray(Wq, np.float32)
    Wk = np.asarray(Wk, np.float32)
    Wv = np.asarray(Wv, np.float32)
    Wo = np.asarray(Wo, np.float32)
    in_maps = []
    for c in range(N_CORES):
        sl = slice(c * DPC, (c + 1) * DPC)
        in_maps.append({
            "xT": xT,
            "wqT": np.ascontiguousarray(Wq[sl, :].T),
            "wkT": np.ascontiguousarray(Wk[sl, :].T),
            "wvT": np.ascontiguousarray(Wv[sl, :].T),
            "woT": np.ascontiguousarray(Wo[:, sl].T),
            "tri": tri,
        })
    res = bass_utils.run_bass_kernel_spmd(
        nc, in_maps, core_ids=list(range(N_CORES)), trace=trace)

    out = np.zeros((NTOK, D), np.float32)
    k = np.empty((NTOK, D), np.float32)
    v = np.empty((NTOK, D), np.float32)
    for c, r in enumerate(res.results):
        sl = slice(c * DPC, (c + 1) * DPC)
        out += np.asarray(r["out_p"]).astype(np.float32)
        k[:, sl] = np.asarray(r["kT_out"]).T
        v[:, sl] = np.asarray(r["v_out"])
    out += np.asarray(bo, np.float32)[None, :]
    outs = (out.reshape(B, T, D), k.reshape(B, T, D), v.reshape(B, T, D))
    return outs, res


def kernel(x, Wq, Wk, Wv, Wo, bo):
    outs, _ = _run(x, Wq, Wk, Wv, Wo, bo, trace=False)
    return outs


# revision 20
# speedup vs baseline: 1.0537x; 1.0077x over previous
"""Trainium2 Bass kernel for nn_MultiHeadAttention (B=4, T=2048, D=2048, H=16).

Sharding: tensor-parallel over heads. Each of 8 NeuronCores owns 2 heads
(256 of the 2048 Q/K/V dims). Matmuls run as float32r (measured ~230ns
sustained at N=512 across 8 cores, vs ~260ns for bf16 whose FWL weight
path is slower in-kernel).

Phase A (projections, all tokens): qT per head in [head_dim, tokens]
layout to a DRAM scratch, kT/v to HBM (kernel outputs). Weight loads are
staged (first Wq chunk -> first x tile -> rest; Wk/Wv on the ACT hwdge
queue) so the PE starts ~8us in.

Phase B (attention): scoresT[ktok, qtok] per 128-ktok block, three
blocks per PSUM group so exp runs as one [128,<=1536] ACT instruction
(amortizes the ~352-cycle ACT fixed cost and keeps ACT ~15% under the
PE rate); causal handling = block skip + triangle-mask multiply on the
diagonal 128x128 + column-sliced score/AV/den matmuls on diagonal
blocks. Softmax denominator via ones-matmul PSUM accumulation;
reciprocal_approx_fast. Normalized ctx for all batches stays in SBUF.

Phase C (output projection): MM-bound block stream, PSUM->SBUF copies
alternating DVE/ACT, out_p written bf16 (pure output, halves write DMA).

Host: Wo partials summed across cores in fp32; k/v slices concatenated.
"""

import os
import sys

import numpy as np

for _p in ("/opt/trn_rl_repo",):
    if _p not in sys.path and os.path.isdir(_p):
        sys.path.insert(0, _p)

B, T, D, H = 4, 2048, 2048, 16
HD = 128
N_CORES = 8
HPC = H // N_CORES          # heads per core
DPC = HPC * HD              # q/k/v dims per core
NTOK = B * T

P = 128
QT = 512                    # q-tile width
KC = 128                    # k-block
PT = 512                    # phase-A token tile
DSUB = 4                    # d-chunks per streamed xT tile
DIAG = QT // KC             # diagonal blocks per q-tile
G = 2                       # max k-blocks per exp group

_CACHE = {}


def _build_module():
    import concourse.bass as bass  # noqa: F401
    import concourse.mybir as mybir
    from concourse import bacc
    import concourse.tile as tile

    F32 = mybir.dt.float32
    F32R = mybir.dt.float32r
    BF16 = mybir.dt.bfloat16
    AF = mybir.ActivationFunctionType

    DK = D // P                 # 16 d-chunks
    NPT = NTOK // PT            # 16 phase-A tiles
    NQT = T // QT               # 4 q-tiles per batch
    TPB = T // P                # 16 token blocks per batch
    NOD = D // QT               # 4 output column chunks
    SCALE = 1.0 / float(np.sqrt(HD))

    nc = bacc.Bacc("TRN2", target_bir_lowering=False, debug=False)

    xT = nc.dram_tensor("xT", [D, NTOK], F32, kind="ExternalInput").ap()
    wqT = nc.dram_tensor("wqT", [D, DPC], F32, kind="ExternalInput").ap()
    wkT = nc.dram_tensor("wkT", [D, DPC], F32, kind="ExternalInput").ap()
    wvT = nc.dram_tensor("wvT", [D, DPC], F32, kind="ExternalInput").ap()
    woT = nc.dram_tensor("woT", [DPC, D], F32, kind="ExternalInput").ap()
    tri = nc.dram_tensor("tri", [KC, KC], F32, kind="ExternalInput").ap()

    kT_out = nc.dram_tensor("kT_out", [DPC, NTOK], F32, kind="ExternalOutput").ap()
    v_out = nc.dram_tensor("v_out", [NTOK, DPC], F32, kind="ExternalOutput").ap()
    out_p = nc.dram_tensor("out_p", [NTOK, D], BF16, kind="ExternalOutput").ap()

    xT_v = xT.rearrange("(dk p) t -> p dk t", p=P)
    wqT_v = wqT.rearrange("(dk p) n -> p dk n", p=P)
    wkT_v = wkT.rearrange("(dk p) n -> p dk n", p=P)
    wvT_v = wvT.rearrange("(dk p) n -> p dk n", p=P)
    woT_v = woT.rearrange("(hc p) n -> p hc n", p=P)
    v_out_v = v_out.rearrange("(c p) n -> p c n", p=P)

    def exp_groups(nkc):
        """Split nkc blocks into groups of at most G."""
        out = []
        kc = 0
        while kc < nkc:
            n = min(G, nkc - kc)
            out.append((kc, n))
            kc += n
        return out

    with tile.TileContext(nc) as tc:
        with (
            tc.tile_pool(name="dram", bufs=1, space="DRAM") as dpool,
            tc.tile_pool(name="pre", bufs=1) as pre_pool,
        ):
            q_scr = dpool.tile([HPC, P, NTOK], F32)
            # pair (0,0) tiles, preloaded during phase A so phase B's first
            # matmul isn't stuck behind the phase-A SP DMA queue
            q0_t = pre_pool.tile([P, T], F32R, tag="q0")
            k0_t = pre_pool.tile([P, T], F32R, tag="k0")
            v0_t = pre_pool.tile([P, T // P, HD], F32R, tag="v0")

            # ---------------- Phase A: projections ----------------
            with (
                tc.tile_pool(name="wq", bufs=1) as wq_pool,
                tc.tile_pool(name="xt", bufs=2 * (DK // DSUB)) as xt_pool,
                tc.tile_pool(name="stA", bufs=4) as stA_pool,
                tc.tile_pool(name="pp_qk", bufs=2, space="PSUM") as pp_qk,
                tc.tile_pool(name="pp_v", bufs=2, space="PSUM") as pp_v,
            ):
                def load_xt(tb):
                    ts = slice(tb * PT, (tb + 1) * PT)
                    xts = []
                    for dg in range(DK // DSUB):
                        xt_t = xt_pool.tile([P, DSUB, PT], F32R, tag="xt")
                        nc.sync.dma_start(
                            xt_t[:],
                            xT_v[:, dg * DSUB:(dg + 1) * DSUB, ts].bitcast(F32R))
                        xts.append(xt_t)
                    return xts

                wq_sb = wq_pool.tile([P, DK, DPC], F32R, tag="wq")
                wk_sb = wq_pool.tile([P, DK, DPC], F32R, tag="wk")
                wv_sb = wq_pool.tile([P, DK, DPC], F32R, tag="wv")
                nc.sync.dma_start(wq_sb[:], wqT_v.bitcast(F32R))
                xts0 = load_xt(0)
                hk = DK // 2
                nc.sync.dma_start(
                    wk_sb[:, 0:hk, :], wkT_v[:, 0:hk, :].bitcast(F32R))
                nc.sync.dma_start(
                    wk_sb[:, hk:DK, :], wkT_v[:, hk:DK, :].bitcast(F32R))
                nc.sync.dma_start(
                    wv_sb[:, 0:hk, :], wvT_v[:, 0:hk, :].bitcast(F32R))
                nc.sync.dma_start(
                    wv_sb[:, hk:DK, :], wvT_v[:, hk:DK, :].bitcast(F32R))

                for tb in range(NPT):
                    xts = xts0 if tb == 0 else load_xt(tb)
                    ts = slice(tb * PT, (tb + 1) * PT)

                    def xchunk(dc):
                        return xts[dc // DSUB][:, dc % DSUB, :]

                    for w_sb, is_q in ((wq_sb, True), (wk_sb, False)):
                        for hc in range(HPC):
                            ps = pp_qk.tile([P, PT], F32, tag="pqk")
                            for dc in range(DK):
                                nc.tensor.matmul(
                                    ps[:],
                                    w_sb[:, dc, hc * P:(hc + 1) * P],
                                    xchunk(dc),
                                    start=(dc == 0), stop=(dc == DK - 1))
                            st = stA_pool.tile([P, PT], F32, tag="stqk")
                            if is_q:
                                nc.vector.tensor_copy(st[:], ps[:])
                                nc.sync.dma_start(q_scr[hc, :, ts], st[:])
                            else:
                                nc.scalar.copy(st[:], ps[:])
                                nc.sync.dma_start(
                                    kT_out[hc * P:(hc + 1) * P, ts], st[:])

                    for sub in range(PT // P):
                        t0 = tb * PT + sub * P
                        ps = pp_v.tile([P, DPC], F32, tag="pv")
                        for dc in range(DK):
                            nc.tensor.matmul(
                                ps[:],
                                xchunk(dc)[:, sub * P:(sub + 1) * P],
                                wv_sb[:, dc, :],
                                start=(dc == 0), stop=(dc == DK - 1))
                        st = stA_pool.tile([P, DPC], F32, tag="stv")
                        if sub % 2 == 0:
                            nc.scalar.copy(st[:], ps[:])
                        else:
                            nc.vector.tensor_copy(st[:], ps[:])
                        nc.sync.dma_start(v_out[t0:t0 + P, :], st[:])

                    if tb == 3:
                        # batch 0 fully projected: prefetch pair (0,0)
                        nc.sync.dma_start(
                            q0_t[:], q_scr[0, :, 0:T].bitcast(F32R))
                        nc.sync.dma_start(
                            k0_t[:], kT_out[0:P, 0:T].bitcast(F32R))
                        nc.sync.dma_start(
                            v0_t[:],
                            v_out_v[:, 0:TPB, 0:HD].bitcast(F32R))

            # -------- Phases B (attention) + C (out-proj) --------
            with (
                tc.tile_pool(name="perB", bufs=1) as perB_pool,
                tc.tile_pool(name="ctx", bufs=B) as ctx_pool,
                tc.tile_pool(name="pair", bufs=2) as pair_pool,
                tc.tile_pool(name="exp", bufs=3) as exp_pool,
                tc.tile_pool(name="rcp", bufs=2) as rcp_pool,
                tc.tile_pool(name="ost", bufs=3) as ost_pool,
            ):
                wo_sb = perB_pool.tile([P, HPC, D], F32R, tag="wo")
                mask_sb = perB_pool.tile([P, KC], F32R, tag="mask")
                ones_f = perB_pool.tile([P, P], F32, tag="onesf")
                ones_sb = perB_pool.tile([P, P], F32R, tag="ones")
                nc.sync.dma_start(mask_sb[:], tri.bitcast(F32R))
                nc.vector.memset(ones_f[:], 1.0)
                nc.vector.tensor_copy(ones_sb[:], ones_f[:])

                ctx_ts = []
                with (
                    tc.tile_pool(name="pp_s", bufs=2, space="PSUM") as pp_s,
                    tc.tile_pool(name="pp_ctx", bufs=2, space="PSUM") as pp_ctx,
                    tc.tile_pool(name="pp_den", bufs=2, space="PSUM") as pp_den,
                ):
                    # initialize the two score PSUM buffers once: sliced
                    # diagonal score matmuls leave stale columns that exp
                    # reads (their e values are never consumed downstream)
                    for _ in range(2):
                        s_init = pp_s.tile([P, G, QT], F32, tag="ps")
                        nc.vector.memset(s_init[:], 0.0)

                    def do_pair(b, h, ctx_t):
                        if b == 0 and h == 0:
                            qt_pair, kt_pair, v_pair = q0_t, k0_t, v0_t
                        else:
                            qt_pair = pair_pool.tile([P, T], F32R, tag="qpair")
                            kt_pair = pair_pool.tile([P, T], F32R, tag="kpair")
                            v_pair = pair_pool.tile(
                                [P, TPB, HD], F32R, tag="vpair")
                            bs = slice(b * T, (b + 1) * T)
                            nc.sync.dma_start(
                                qt_pair[:], q_scr[h, :, bs].bitcast(F32R))
                            nc.sync.dma_start(
                                kt_pair[:],
                                kT_out[h * P:(h + 1) * P, bs].bitcast(F32R))
                            nc.sync.dma_start(
                                v_pair[:],
                                v_out_v[:, b * TPB:(b + 1) * TPB,
                                        h * HD:(h + 1) * HD].bitcast(F32R))

                        for qt in range(NQT):
                            nkc = (qt + 1) * DIAG
                            ctx_ps = pp_ctx.tile([P, QT], F32, tag="pctx")
                            den_ps = pp_den.tile([P, QT], F32, tag="pden")
                            for kc0, gn in exp_groups(nkc):
                                s_ps = pp_s.tile([P, G, QT], F32, tag="ps")
                                for u in range(gn):
                                    kc = kc0 + u
                                    j = kc - qt * DIAG
                                    lo = j * KC if j >= 0 else 0
                                    nc.tensor.matmul(
                                        s_ps[:, u, lo:],
                                        kt_pair[:, kc * KC:(kc + 1) * KC],
                                        qt_pair[:, qt * QT + lo:(qt + 1) * QT],
                                        start=True, stop=True,
                                        skip_group_check=True)
                                e_g = exp_pool.tile([P, G, QT], F32R, tag="eg")
                                nc.scalar.activation(
                                    e_g[:, 0:gn, :], s_ps[:, 0:gn, :],
                                    AF.Exp, scale=SCALE)
                                for u in range(gn):
                                    kc = kc0 + u
                                    j = kc - qt * DIAG
                                    lo = j * KC if j >= 0 else 0
                                    if j >= 0:
                                        # GpSimd: off the DVE critical path
                                        # (recip+norm of the previous q-tile)
                                        nc.gpsimd.tensor_mul(
                                            e_g[:, u, j * KC:(j + 1) * KC],
                                            e_g[:, u, j * KC:(j + 1) * KC],
                                            mask_sb[:])
                                    nc.tensor.matmul(
                                        ctx_ps[:, lo:],
                                        v_pair[:, kc, :],
                                        e_g[:, u, lo:],
                                        start=(kc == 0), stop=(kc == nkc - 1),
                                        skip_group_check=True)
                                    nc.tensor.matmul(
                                        den_ps[:, lo:],
                                        ones_sb[:],
                                        e_g[:, u, lo:],
                                        start=(kc == 0), stop=(kc == nkc - 1),
                                        skip_group_check=True)
                            rcp = rcp_pool.tile([P, QT], F32, tag="rcp")
                            nc.vector.reciprocal_approx_fast(rcp[:], den_ps[:])
                            nc.vector.tensor_mul(
                                ctx_t[:, h, qt * QT:(qt + 1) * QT],
                                ctx_ps[:], rcp[:])

                    for b in range(B):
                        if b == 1:
                            # wo arrives during attention, ready for phase C
                            nc.sync.dma_start(wo_sb[:], woT_v.bitcast(F32R))
                        ctx_t = ctx_pool.tile([P, HPC, T], F32R, tag="ctx")
                        ctx_ts.append(ctx_t)
                        for h in range(HPC):
                            do_pair(b, h, ctx_t)

                # ---------------- Phase C: output projection ----------------
                with tc.tile_pool(name="pp_o", bufs=4, space="PSUM") as pp_o:
                    ncopy = 0
                    for b in range(B):
                        ctx_t = ctx_ts[b]
                        for tb2 in range(TPB):
                            t0 = b * T + tb2 * P
                            ost = ost_pool.tile([P, D], BF16, tag="ost")
                            for od in range(NOD):
                                ods = slice(od * QT, (od + 1) * QT)
                                ps0 = pp_o.tile([P, QT], F32, tag="po")
                                nc.tensor.matmul(
                                    ps0[:], ctx_t[:, 0, tb2 * P:(tb2 + 1) * P],
                                    wo_sb[:, 0, ods], start=True, stop=False)
                                nc.tensor.matmul(
                                    ps0[:], ctx_t[:, 1, tb2 * P:(tb2 + 1) * P],
                                    wo_sb[:, 1, ods], start=False, stop=True)
                                if ncopy % 2 == 0:
                                    nc.vector.tensor_copy(ost[:, ods], ps0[:])
                                else:
                                    nc.scalar.copy(ost[:, ods], ps0[:])
                                ncopy += 1
                            nc.sync.dma_start(out_p[t0:t0 + P, :], ost[:])

    nc.compile()
    return nc


def _get_module():
    if "nc" not in _CACHE:
        _CACHE["nc"] = _build_module()
    return _CACHE["nc"]


def _make_tri():
    return (np.arange(KC)[:, None] <= np.arange(KC)[None, :]).astype(np.float32)


def _run(x, Wq, Wk, Wv, Wo, bo, trace=False):
    from concourse import bass_utils

    nc = _get_module()
    x = np.asarray(x, dtype=np.float32)
    xT = np.ascontiguousarray(x.reshape(NTOK, D).T)
    tri = _make_tri()
    Wq = np.asarray(Wq, np.float32)
    Wk = np.asarray(Wk, np.float32)
    Wv = np.asarray(Wv, np.float32)
    Wo = np.asarray(Wo, np.float32)
    in_maps = []
    for c in range(N_CORES):
        sl = slice(c * DPC, (c + 1) * DPC)
        in_maps.append({
            "xT": xT,
            "wqT": np.ascontiguousarray(Wq[sl, :].T),
            "wkT": np.ascontiguousarray(Wk[sl, :].T),
            "wvT": np.ascontiguousarray(Wv[sl, :].T),
            "woT": np.ascontiguousarray(Wo[:, sl].T),
            "tri": tri,
        })
    res = bass_utils.run_bass_kernel_spmd(
        nc, in_maps, core_ids=list(range(N_CORES)), trace=trace)

    out = np.zeros((NTOK, D), np.float32)
    k = np.empty((NTOK, D), np.float32)
    v = np.empty((NTOK, D), np.float32)
    for c, r in enumerate(res.results):
        sl = slice(c * DPC, (c + 1) * DPC)
        out += np.asarray(r["out_p"]).astype(np.float32)
        k[:, sl] = np.asarray(r["kT_out"]).T
        v[:, sl] = np.asarray(r["v_out"])
    out += np.asarray(bo, np.float32)[None, :]
    outs = (out.reshape(B, T, D), k.reshape(B, T, D), v.reshape(B, T, D))
    return outs, res


def kernel(x, Wq, Wk, Wv, Wo, bo):
    outs, _ = _run(x, Wq, Wk, Wv, Wo, bo, trace=False)
    return outs


# revision 22
# speedup vs baseline: 1.0670x; 1.0126x over previous
"""Trainium2 Bass kernel for nn_MultiHeadAttention (B=4, T=2048, D=2048, H=16).

Sharding: tensor-parallel over heads. Each of 8 NeuronCores owns 2 heads
(256 of the 2048 Q/K/V dims). Matmuls run as float32r (measured ~230ns
sustained at N=512 across 8 cores, vs ~260ns for bf16 whose FWL weight
path is slower in-kernel).

Phase A (projections, all tokens): qT per head in [head_dim, tokens]
layout to a DRAM scratch, kT/v to HBM (kernel outputs). Weight loads are
staged (first Wq chunk -> first x tile -> rest; Wk/Wv on the ACT hwdge
queue) so the PE starts ~8us in.

Phase B (attention): scoresT[ktok, qtok] per 128-ktok block, three
blocks per PSUM group so exp runs as one [128,<=1536] ACT instruction
(amortizes the ~352-cycle ACT fixed cost and keeps ACT ~15% under the
PE rate); causal handling = block skip + triangle-mask multiply on the
diagonal 128x128 + column-sliced score/AV/den matmuls on diagonal
blocks. Softmax denominator via ones-matmul PSUM accumulation;
reciprocal_approx_fast. Normalized ctx for all batches stays in SBUF.

Phase C (output projection): MM-bound block stream, PSUM->SBUF copies
alternating DVE/ACT, out_p written bf16 (pure output, halves write DMA).

Host: Wo partials summed across cores in fp32; k/v slices concatenated.
"""

import os
import sys

import numpy as np

for _p in ("/opt/trn_rl_repo",):
    if _p not in sys.path and os.path.isdir(_p):
        sys.path.insert(0, _p)

B, T, D, H = 4, 2048, 2048, 16
HD = 128
N_CORES = 8
HPC = H // N_CORES          # heads per core
DPC = HPC * HD              # q/k/v dims per core
NTOK = B * T

P = 128
QT = 512                    # q-tile width
KC = 128                    # k-block
PT = 512                    # phase-A token tile
DSUB = 4                    # d-chunks per streamed xT tile
DIAG = QT // KC             # diagonal blocks per q-tile
G = 2                       # max k-blocks per exp group

_CACHE = {}


def _build_module():
    import concourse.bass as bass  # noqa: F401
    import concourse.mybir as mybir
    from concourse import bacc
    import concourse.tile as tile

    F32 = mybir.dt.float32
    F32R = mybir.dt.float32r
    BF16 = mybir.dt.bfloat16
    AF = mybir.ActivationFunctionType

    DK = D // P                 # 16 d-chunks
    NPT = NTOK // PT            # 16 phase-A tiles
    NQT = T // QT               # 4 q-tiles per batch
    TPB = T // P                # 16 token blocks per batch
    NOD = D // QT               # 4 output column chunks
    SCALE = 1.0 / float(np.sqrt(HD))

    nc = bacc.Bacc("TRN2", target_bir_lowering=False, debug=False)

    xT = nc.dram_tensor("xT", [D, NTOK], F32, kind="ExternalInput").ap()
    wqT = nc.dram_tensor("wqT", [D, DPC], F32, kind="ExternalInput").ap()
    wkT = nc.dram_tensor("wkT", [D, DPC], F32, kind="ExternalInput").ap()
    wvT = nc.dram_tensor("wvT", [D, DPC], F32, kind="ExternalInput").ap()
    woT = nc.dram_tensor("woT", [DPC, D], F32, kind="ExternalInput").ap()
    tri = nc.dram_tensor("tri", [KC, KC], F32, kind="ExternalInput").ap()

    kT_out = nc.dram_tensor("kT_out", [DPC, NTOK], F32, kind="ExternalOutput").ap()
    v_out = nc.dram_tensor("v_out", [NTOK, DPC], F32, kind="ExternalOutput").ap()
    out_p = nc.dram_tensor("out_p", [NTOK, D], BF16, kind="ExternalOutput").ap()

    xT_v = xT.rearrange("(dk p) t -> p dk t", p=P)
    wqT_v = wqT.rearrange("(dk p) n -> p dk n", p=P)
    wkT_v = wkT.rearrange("(dk p) n -> p dk n", p=P)
    wvT_v = wvT.rearrange("(dk p) n -> p dk n", p=P)
    woT_v = woT.rearrange("(hc p) n -> p hc n", p=P)
    v_out_v = v_out.rearrange("(c p) n -> p c n", p=P)

    def exp_groups(nkc):
        """Split nkc blocks into groups of at most G."""
        out = []
        kc = 0
        while kc < nkc:
            n = min(G, nkc - kc)
            out.append((kc, n))
            kc += n
        return out

    with tile.TileContext(nc) as tc:
        with (
            tc.tile_pool(name="dram", bufs=1, space="DRAM") as dpool,
            tc.tile_pool(name="pre", bufs=1) as pre_pool,
        ):
            q_scr = dpool.tile([HPC, P, NTOK], F32)
            # pair (0,0) tiles, preloaded during phase A so phase B's first
            # matmul isn't stuck behind the phase-A SP DMA queue
            q0_t = pre_pool.tile([P, T], F32R, tag="q0")
            k0_t = pre_pool.tile([P, T], F32R, tag="k0")
            v0_t = pre_pool.tile([P, T // P, HD], F32R, tag="v0")

            # ---------------- Phase A: projections ----------------
            with (
                tc.tile_pool(name="wq", bufs=1) as wq_pool,
                tc.tile_pool(name="xt", bufs=2 * (DK // DSUB)) as xt_pool,
                tc.tile_pool(name="stA", bufs=4) as stA_pool,
                tc.tile_pool(name="pp_qk", bufs=2, space="PSUM") as pp_qk,
                tc.tile_pool(name="pp_v", bufs=2, space="PSUM") as pp_v,
            ):
                def load_xt(tb):
                    ts = slice(tb * PT, (tb + 1) * PT)
                    xts = []
                    for dg in range(DK // DSUB):
                        xt_t = xt_pool.tile([P, DSUB, PT], F32R, tag="xt")
                        nc.sync.dma_start(
                            xt_t[:],
                            xT_v[:, dg * DSUB:(dg + 1) * DSUB, ts].bitcast(F32R))
                        xts.append(xt_t)
                    return xts

                wq_sb = wq_pool.tile([P, DK, DPC], F32R, tag="wq")
                wk_sb = wq_pool.tile([P, DK, DPC], F32R, tag="wk")
                wv_sb = wq_pool.tile([P, DK, DPC], F32R, tag="wv")
                nc.sync.dma_start(wq_sb[:], wqT_v.bitcast(F32R))
                xts0 = load_xt(0)
                nc.sync.dma_start(wk_sb[:], wkT_v.bitcast(F32R))
                nc.sync.dma_start(wv_sb[:], wvT_v.bitcast(F32R))

                for tb in range(NPT):
                    xts = xts0 if tb == 0 else load_xt(tb)
                    ts = slice(tb * PT, (tb + 1) * PT)

                    def xchunk(dc):
                        return xts[dc // DSUB][:, dc % DSUB, :]

                    for w_sb, is_q in ((wq_sb, True), (wk_sb, False)):
                        for hc in range(HPC):
                            ps = pp_qk.tile([P, PT], F32, tag="pqk")
                            for dc in range(DK):
                                nc.tensor.matmul(
                                    ps[:],
                                    w_sb[:, dc, hc * P:(hc + 1) * P],
                                    xchunk(dc),
                                    start=(dc == 0), stop=(dc == DK - 1))
                            st = stA_pool.tile([P, PT], F32, tag="stqk")
                            if is_q:
                                nc.vector.tensor_copy(st[:], ps[:])
                                nc.sync.dma_start(q_scr[hc, :, ts], st[:])
                            else:
                                nc.scalar.copy(st[:], ps[:])
                                nc.sync.dma_start(
                                    kT_out[hc * P:(hc + 1) * P, ts], st[:])

                    for sub in range(PT // P):
                        t0 = tb * PT + sub * P
                        ps = pp_v.tile([P, DPC], F32, tag="pv")
                        for dc in range(DK):
                            nc.tensor.matmul(
                                ps[:],
                                xchunk(dc)[:, sub * P:(sub + 1) * P],
                                wv_sb[:, dc, :],
                                start=(dc == 0), stop=(dc == DK - 1))
                        st = stA_pool.tile([P, DPC], F32, tag="stv")
                        if sub % 2 == 0:
                            nc.scalar.copy(st[:], ps[:])
                        else:
                            nc.vector.tensor_copy(st[:], ps[:])
                        nc.sync.dma_start(v_out[t0:t0 + P, :], st[:])

                    if tb == 3:
                        # batch 0 fully projected: prefetch pair (0,0) on
                        # the ACT hwdge queue (slow but idle; ~250us of
                        # runway) so SP's x-tile prefetches aren't delayed
                        nc.scalar.dma_start(
                            q0_t[:], q_scr[0, :, 0:T].bitcast(F32R))
                        nc.scalar.dma_start(
                            k0_t[:], kT_out[0:P, 0:T].bitcast(F32R))
                        nc.scalar.dma_start(
                            v0_t[:],
                            v_out_v[:, 0:TPB, 0:HD].bitcast(F32R))

            # -------- Phases B (attention) + C (out-proj) --------
            with (
                tc.tile_pool(name="perB", bufs=1) as perB_pool,
                tc.tile_pool(name="ctx", bufs=B) as ctx_pool,
                tc.tile_pool(name="pair", bufs=2) as pair_pool,
                tc.tile_pool(name="exp", bufs=3) as exp_pool,
                tc.tile_pool(name="rcp", bufs=2) as rcp_pool,
                tc.tile_pool(name="ost", bufs=3) as ost_pool,
            ):
                wo_sb = perB_pool.tile([P, HPC, D], F32R, tag="wo")
                mask_sb = perB_pool.tile([P, KC], F32R, tag="mask")
                ones_f = perB_pool.tile([P, P], F32, tag="onesf")
                ones_sb = perB_pool.tile([P, P], F32R, tag="ones")
                nc.sync.dma_start(mask_sb[:], tri.bitcast(F32R))
                nc.vector.memset(ones_f[:], 1.0)
                nc.vector.tensor_copy(ones_sb[:], ones_f[:])

                ctx_ts = []
                with (
                    tc.tile_pool(name="pp_s", bufs=2, space="PSUM") as pp_s,
                    tc.tile_pool(name="pp_ctx", bufs=2, space="PSUM") as pp_ctx,
                    tc.tile_pool(name="pp_den", bufs=2, space="PSUM") as pp_den,
                ):
                    # initialize the two score PSUM buffers once: sliced
                    # diagonal score matmuls leave stale columns that exp
                    # reads (their e values are never consumed downstream)
                    for _ in range(2):
                        s_init = pp_s.tile([P, G, QT], F32, tag="ps")
                        nc.vector.memset(s_init[:], 0.0)

                    def do_pair(b, h, ctx_t):
                        if b == 0 and h == 0:
                            qt_pair, kt_pair, v_pair = q0_t, k0_t, v0_t
                        else:
                            qt_pair = pair_pool.tile([P, T], F32R, tag="qpair")
                            kt_pair = pair_pool.tile([P, T], F32R, tag="kpair")
                            v_pair = pair_pool.tile(
                                [P, TPB, HD], F32R, tag="vpair")
                            bs = slice(b * T, (b + 1) * T)
                            nc.sync.dma_start(
                                qt_pair[:], q_scr[h, :, bs].bitcast(F32R))
                            nc.sync.dma_start(
                                kt_pair[:],
                                kT_out[h * P:(h + 1) * P, bs].bitcast(F32R))
                            nc.sync.dma_start(
                                v_pair[:],
                                v_out_v[:, b * TPB:(b + 1) * TPB,
                                        h * HD:(h + 1) * HD].bitcast(F32R))

                        for qt in range(NQT):
                            nkc = (qt + 1) * DIAG
                            ctx_ps = pp_ctx.tile([P, QT], F32, tag="pctx")
                            den_ps = pp_den.tile([P, QT], F32, tag="pden")
                            for kc0, gn in exp_groups(nkc):
                                s_ps = pp_s.tile([P, G, QT], F32, tag="ps")
                                for u in range(gn):
                                    kc = kc0 + u
                                    j = kc - qt * DIAG
                                    lo = j * KC if j >= 0 else 0
                                    nc.tensor.matmul(
                                        s_ps[:, u, lo:],
                                        kt_pair[:, kc * KC:(kc + 1) * KC],
                                        qt_pair[:, qt * QT + lo:(qt + 1) * QT],
                                        start=True, stop=True,
                                        skip_group_check=True)
                                e_g = exp_pool.tile([P, G, QT], F32R, tag="eg")
                                nc.scalar.activation(
                                    e_g[:, 0:gn, :], s_ps[:, 0:gn, :],
                                    AF.Exp, scale=SCALE)
                                for u in range(gn):
                                    kc = kc0 + u
                                    j = kc - qt * DIAG
                                    lo = j * KC if j >= 0 else 0
                                    if j >= 0:
                                        # GpSimd: off the DVE critical path
                                        # (recip+norm of the previous q-tile)
                                        nc.gpsimd.tensor_mul(
                                            e_g[:, u, j * KC:(j + 1) * KC],
                                            e_g[:, u, j * KC:(j + 1) * KC],
                                            mask_sb[:])
                                    nc.tensor.matmul(
                                        ctx_ps[:, lo:],
                                        v_pair[:, kc, :],
                                        e_g[:, u, lo:],
                                        start=(kc == 0), stop=(kc == nkc - 1),
                                        skip_group_check=True)
                                    nc.tensor.matmul(
                                        den_ps[:, lo:],
                                        ones_sb[:],
                                        e_g[:, u, lo:],
                                        start=(kc == 0), stop=(kc == nkc - 1),
                                        skip_group_check=True)
                            rcp = rcp_pool.tile([P, QT], F32, tag="rcp")
                            nc.vector.reciprocal_approx_fast(rcp[:], den_ps[:])
                            nc.vector.tensor_mul(
                                ctx_t[:, h, qt * QT:(qt + 1) * QT],
                                ctx_ps[:], rcp[:])

                    for b in range(B):
                        if b == 1:
                            # wo arrives during attention, ready for phase C
                            nc.sync.dma_start(wo_sb[:], woT_v.bitcast(F32R))
                        ctx_t = ctx_pool.tile([P, HPC, T], F32R, tag="ctx")
                        ctx_ts.append(ctx_t)
                        for h in range(HPC):
                            do_pair(b, h, ctx_t)

                # ---------------- Phase C: output projection ----------------
                with tc.tile_pool(name="pp_o", bufs=4, space="PSUM") as pp_o:
                    ncopy = 0
                    for b in range(B):
                        ctx_t = ctx_ts[b]
                        for tb2 in range(TPB):
                            t0 = b * T + tb2 * P
                            ost = ost_pool.tile([P, D], BF16, tag="ost")
                            for od in range(NOD):
                                ods = slice(od * QT, (od + 1) * QT)
                                ps0 = pp_o.tile([P, QT], F32, tag="po")
                                nc.tensor.matmul(
                                    ps0[:], ctx_t[:, 0, tb2 * P:(tb2 + 1) * P],
                                    wo_sb[:, 0, ods], start=True, stop=False)
                                nc.tensor.matmul(
                                    ps0[:], ctx_t[:, 1, tb2 * P:(tb2 + 1) * P],
                                    wo_sb[:, 1, ods], start=False, stop=True)
                                if ncopy % 2 == 0:
                                    nc.vector.tensor_copy(ost[:, ods], ps0[:])
                                else:
                                    nc.scalar.copy(ost[:, ods], ps0[:])
                                ncopy += 1
                            nc.sync.dma_start(out_p[t0:t0 + P, :], ost[:])

    nc.compile()
    return nc


def _get_module():
    if "nc" not in _CACHE:
        _CACHE["nc"] = _build_module()
    return _CACHE["nc"]


def _make_tri():
    return (np.arange(KC)[:, None] <= np.arange(KC)[None, :]).astype(np.float32)


def _run(x, Wq, Wk, Wv, Wo, bo, trace=False):
    from concourse import bass_utils

    nc = _get_module()
    x = np.asarray(x, dtype=np.float32)
    xT = np.ascontiguousarray(x.reshape(NTOK, D).T)
    tri = _make_tri()
    Wq = np.asarray(Wq, np.float32)
    Wk = np.asarray(Wk, np.float32)
    Wv = np.asarray(Wv, np.float32)
    Wo = np.asarray(Wo, np.float32)
    in_maps = []
    for c in range(N_CORES):
        sl = slice(c * DPC, (c + 1) * DPC)
        in_maps.append({
            "xT": xT,
            "wqT": np.ascontiguousarray(Wq[sl, :].T),
            "wkT": np.ascontiguousarray(Wk[sl, :].T),
            "wvT": np.ascontiguousarray(Wv[sl, :].T),
            "woT": np.ascontiguousarray(Wo[:, sl].T),
            "tri": tri,
        })
    res = bass_utils.run_bass_kernel_spmd(
        nc, in_maps, core_ids=list(range(N_CORES)), trace=trace)

    out = np.zeros((NTOK, D), np.float32)
    k = np.empty((NTOK, D), np.float32)
    v = np.empty((NTOK, D), np.float32)
    for c, r in enumerate(res.results):
        sl = slice(c * DPC, (c + 1) * DPC)
        out += np.asarray(r["out_p"]).astype(np.float32)
        k[:, sl] = np.asarray(r["kT_out"]).T
        v[:, sl] = np.asarray(r["v_out"])
    out += np.asarray(bo, np.float32)[None, :]
    outs = (out.reshape(B, T, D), k.reshape(B, T, D), v.reshape(B, T, D))
    return outs, res


def kernel(x, Wq, Wk, Wv, Wo, bo):
    outs, _ = _run(x, Wq, Wk, Wv, Wo, bo, trace=False)
    return outs


# revision 23
# speedup vs baseline: 1.0828x; 1.0149x over previous
"""Trainium2 Bass kernel for nn_MultiHeadAttention (B=4, T=2048, D=2048, H=16).

Sharding: tensor-parallel over heads. Each of 8 NeuronCores owns 2 heads
(256 of the 2048 Q/K/V dims). Matmuls run as float32r (measured ~230ns
sustained at N=512 across 8 cores, vs ~260ns for bf16 whose FWL weight
path is slower in-kernel).

Phase A (projections, all tokens): qT per head in [head_dim, tokens]
layout to a DRAM scratch, kT/v to HBM (kernel outputs). The first
attention pair is prefetched during phase A on the ACT hwdge queue so
phase B starts without a DMA stall.

Phase B (attention): scoresT[ktok, qtok] per 128-ktok block, two
blocks per PSUM group so exp runs as one [128,1024] ACT instruction
(amortizes the ~352-cycle ACT fixed cost); causal handling = block
skip + triangle-mask multiply (on GpSimd, off the DVE critical path)
of the diagonal 128x128 + column-sliced score/AV/den matmuls on
diagonal blocks. Softmax denominator via ones-matmul PSUM
accumulation; reciprocal_approx_fast. ctx for all batches stays in
SBUF (f32r).

Phase C (output projection): MM-bound block stream, PSUM->SBUF copies
alternating DVE/ACT, out_p written bf16 (pure output, halves write DMA).

Host: Wo partials summed across cores in fp32; k/v slices concatenated.
"""

import os
import sys

import numpy as np

for _p in ("/opt/trn_rl_repo",):
    if _p not in sys.path and os.path.isdir(_p):
        sys.path.insert(0, _p)

B, T, D, H = 4, 2048, 2048, 16
HD = 128
N_CORES = 8
HPC = H // N_CORES          # heads per core
DPC = HPC * HD              # q/k/v dims per core
NTOK = B * T

P = 128
QT = 512                    # q-tile width
KC = 128                    # k-block
PT = 512                    # phase-A token tile
DSUB = 4                    # d-chunks per streamed xT tile
DIAG = QT // KC             # diagonal blocks per q-tile
G = 2                       # max k-blocks per exp group

_CACHE = {}


def _build_module():
    import concourse.bass as bass  # noqa: F401
    import concourse.mybir as mybir
    from concourse import bacc
    import concourse.tile as tile

    F32 = mybir.dt.float32
    F32R = mybir.dt.float32r
    BF16 = mybir.dt.bfloat16
    AF = mybir.ActivationFunctionType

    DK = D // P                 # 16 d-chunks
    NPT = NTOK // PT            # 16 phase-A tiles
    NQT = T // QT               # 4 q-tiles per batch
    TPB = T // P                # 16 token blocks per batch
    NOD = D // QT               # 4 output column chunks
    SCALE = 1.0 / float(np.sqrt(HD))

    nc = bacc.Bacc("TRN2", target_bir_lowering=False, debug=False)

    xT = nc.dram_tensor("xT", [D, NTOK], F32, kind="ExternalInput").ap()
    wqT = nc.dram_tensor("wqT", [D, DPC], F32, kind="ExternalInput").ap()
    wkT = nc.dram_tensor("wkT", [D, DPC], F32, kind="ExternalInput").ap()
    wvT = nc.dram_tensor("wvT", [D, DPC], F32, kind="ExternalInput").ap()
    woT = nc.dram_tensor("woT", [DPC, D], F32, kind="ExternalInput").ap()
    tri = nc.dram_tensor("tri", [KC, KC], F32, kind="ExternalInput").ap()

    kT_out = nc.dram_tensor("kT_out", [DPC, NTOK], F32, kind="ExternalOutput").ap()
    v_out = nc.dram_tensor("v_out", [NTOK, DPC], F32, kind="ExternalOutput").ap()
    out_p = nc.dram_tensor("out_p", [NTOK, D], BF16, kind="ExternalOutput").ap()

    xT_v = xT.rearrange("(dk p) t -> p dk t", p=P)
    wqT_v = wqT.rearrange("(dk p) n -> p dk n", p=P)
    wkT_v = wkT.rearrange("(dk p) n -> p dk n", p=P)
    wvT_v = wvT.rearrange("(dk p) n -> p dk n", p=P)
    woT_v = woT.rearrange("(hc p) n -> p hc n", p=P)
    v_out_v = v_out.rearrange("(c p) n -> p c n", p=P)

    def exp_groups(nkc):
        """Split nkc blocks into groups of at most G."""
        out = []
        kc = 0
        while kc < nkc:
            n = min(G, nkc - kc)
            out.append((kc, n))
            kc += n
        return out

    with tile.TileContext(nc) as tc:
        with (
            tc.tile_pool(name="dram", bufs=1, space="DRAM") as dpool,
            tc.tile_pool(name="pre", bufs=1) as pre_pool,
        ):
            q_scr = dpool.tile([HPC, P, NTOK], F32)
            # pair (0,0) tiles, preloaded during phase A so phase B's first
            # matmul isn't stuck behind the phase-A SP DMA queue
            q0_t = pre_pool.tile([P, T], F32R, tag="q0")
            k0_t = pre_pool.tile([P, T], F32R, tag="k0")
            v0_t = pre_pool.tile([P, T // P, HD], F32R, tag="v0")

            # ---------------- Phase A: projections ----------------
            with (
                tc.tile_pool(name="wq", bufs=1) as wq_pool,
                tc.tile_pool(name="xt", bufs=2 * (DK // DSUB)) as xt_pool,
                tc.tile_pool(name="stA", bufs=4) as stA_pool,
                tc.tile_pool(name="pp_qk", bufs=2, space="PSUM") as pp_qk,
                tc.tile_pool(name="pp_v", bufs=2, space="PSUM") as pp_v,
            ):
                def load_xt(tb):
                    ts = slice(tb * PT, (tb + 1) * PT)
                    xts = []
                    for dg in range(DK // DSUB):
                        xt_t = xt_pool.tile([P, DSUB, PT], F32R, tag="xt")
                        nc.sync.dma_start(
                            xt_t[:],
                            xT_v[:, dg * DSUB:(dg + 1) * DSUB, ts].bitcast(F32R))
                        xts.append(xt_t)
                    return xts

                wq_sb = wq_pool.tile([P, DK, DPC], F32R, tag="wq")
                wk_sb = wq_pool.tile([P, DK, DPC], F32R, tag="wk")
                wv_sb = wq_pool.tile([P, DK, DPC], F32R, tag="wv")
                nc.sync.dma_start(wq_sb[:], wqT_v.bitcast(F32R))
                xts0 = load_xt(0)
                nc.sync.dma_start(wk_sb[:], wkT_v.bitcast(F32R))
                nc.sync.dma_start(wv_sb[:], wvT_v.bitcast(F32R))

                for tb in range(NPT):
                    xts = xts0 if tb == 0 else load_xt(tb)
                    ts = slice(tb * PT, (tb + 1) * PT)

                    def xchunk(dc):
                        return xts[dc // DSUB][:, dc % DSUB, :]

                    for w_sb, is_q in ((wq_sb, True), (wk_sb, False)):
                        for hc in range(HPC):
                            ps = pp_qk.tile([P, PT], F32, tag="pqk")
                            for dc in range(DK):
                                nc.tensor.matmul(
                                    ps[:],
                                    w_sb[:, dc, hc * P:(hc + 1) * P],
                                    xchunk(dc),
                                    start=(dc == 0), stop=(dc == DK - 1))
                            st = stA_pool.tile([P, PT], F32, tag="stqk")
                            if is_q:
                                nc.vector.tensor_copy(st[:], ps[:])
                                nc.sync.dma_start(q_scr[hc, :, ts], st[:])
                            else:
                                nc.scalar.copy(st[:], ps[:])
                                nc.sync.dma_start(
                                    kT_out[hc * P:(hc + 1) * P, ts], st[:])

                    for sub in range(PT // P):
                        t0 = tb * PT + sub * P
                        ps = pp_v.tile([P, DPC], F32, tag="pv")
                        for dc in range(DK):
                            nc.tensor.matmul(
                                ps[:],
                                xchunk(dc)[:, sub * P:(sub + 1) * P],
                                wv_sb[:, dc, :],
                                start=(dc == 0), stop=(dc == DK - 1))
                        st = stA_pool.tile([P, DPC], F32, tag="stv")
                        if sub % 2 == 0:
                            nc.scalar.copy(st[:], ps[:])
                        else:
                            nc.vector.tensor_copy(st[:], ps[:])
                        nc.sync.dma_start(v_out[t0:t0 + P, :], st[:])

                    if tb == 3:
                        # batch 0 fully projected: prefetch pair (0,0) on
                        # the ACT hwdge queue (slow but idle; ~250us of
                        # runway) so SP's x-tile prefetches aren't delayed
                        nc.scalar.dma_start(
                            q0_t[:], q_scr[0, :, 0:T].bitcast(F32R))
                        nc.scalar.dma_start(
                            k0_t[:], kT_out[0:P, 0:T].bitcast(F32R))
                        nc.scalar.dma_start(
                            v0_t[:],
                            v_out_v[:, 0:TPB, 0:HD].bitcast(F32R))

            # -------- Phases B (attention) + C (out-proj) --------
            with (
                tc.tile_pool(name="perB", bufs=1) as perB_pool,
                tc.tile_pool(name="ctx", bufs=B) as ctx_pool,
                tc.tile_pool(name="pair", bufs=2) as pair_pool,
                tc.tile_pool(name="exp", bufs=3) as exp_pool,
                tc.tile_pool(name="rcp", bufs=2) as rcp_pool,
                tc.tile_pool(name="ost", bufs=3) as ost_pool,
            ):
                wo_sb = perB_pool.tile([P, HPC, D], F32R, tag="wo")
                mask_sb = perB_pool.tile([P, KC], F32R, tag="mask")
                ones_f = perB_pool.tile([P, P], F32, tag="onesf")
                ones_sb = perB_pool.tile([P, P], F32R, tag="ones")
                nc.sync.dma_start(mask_sb[:], tri.bitcast(F32R))
                nc.vector.memset(ones_f[:], 1.0)
                nc.vector.tensor_copy(ones_sb[:], ones_f[:])

                ctx_ts = []
                with (
                    tc.tile_pool(name="pp_s", bufs=2, space="PSUM") as pp_s,
                    tc.tile_pool(name="pp_ctx", bufs=2, space="PSUM") as pp_ctx,
                    tc.tile_pool(name="pp_den", bufs=2, space="PSUM") as pp_den,
                ):
                    # initialize the two score PSUM buffers once: sliced
                    # diagonal score matmuls leave stale columns that exp
                    # reads (their e values are never consumed downstream)
                    for _ in range(2):
                        s_init = pp_s.tile([P, G, QT], F32, tag="ps")
                        nc.vector.memset(s_init[:], 0.0)

                    def do_pair(b, h, ctx_t):
                        if b == 0 and h == 0:
                            qt_pair, kt_pair, v_pair = q0_t, k0_t, v0_t
                        else:
                            qt_pair = pair_pool.tile([P, T], F32R, tag="qpair")
                            kt_pair = pair_pool.tile([P, T], F32R, tag="kpair")
                            v_pair = pair_pool.tile(
                                [P, TPB, HD], F32R, tag="vpair")
                            bs = slice(b * T, (b + 1) * T)
                            nc.sync.dma_start(
                                qt_pair[:], q_scr[h, :, bs].bitcast(F32R))
                            nc.sync.dma_start(
                                kt_pair[:],
                                kT_out[h * P:(h + 1) * P, bs].bitcast(F32R))
                            nc.sync.dma_start(
                                v_pair[:],
                                v_out_v[:, b * TPB:(b + 1) * TPB,
                                        h * HD:(h + 1) * HD].bitcast(F32R))

                        for qt in range(NQT):
                            nkc = (qt + 1) * DIAG
                            ctx_ps = pp_ctx.tile([P, QT], F32, tag="pctx")
                            den_ps = pp_den.tile([P, QT], F32, tag="pden")
                            for kc0, gn in exp_groups(nkc):
                                s_ps = pp_s.tile([P, G, QT], F32, tag="ps")
                                for u in range(gn):
                                    kc = kc0 + u
                                    j = kc - qt * DIAG
                                    lo = j * KC if j >= 0 else 0
                                    nc.tensor.matmul(
                                        s_ps[:, u, lo:],
                                        kt_pair[:, kc * KC:(kc + 1) * KC],
                                        qt_pair[:, qt * QT + lo:(qt + 1) * QT],
                                        start=True, stop=True,
                                        skip_group_check=True)
                                e_g = exp_pool.tile([P, G, QT], F32R, tag="eg")
                                nc.scalar.activation(
                                    e_g[:, 0:gn, :], s_ps[:, 0:gn, :],
                                    AF.Exp, scale=SCALE)
                                for u in range(gn):
                                    kc = kc0 + u
                                    j = kc - qt * DIAG
                                    lo = j * KC if j >= 0 else 0
                                    if j >= 0:
                                        # GpSimd: off the DVE critical path
                                        # (recip+norm of the previous q-tile)
                                        nc.gpsimd.tensor_mul(
                                            e_g[:, u, j * KC:(j + 1) * KC],
                                            e_g[:, u, j * KC:(j + 1) * KC],
                                            mask_sb[:])
                                    nc.tensor.matmul(
                                        ctx_ps[:, lo:],
                                        v_pair[:, kc, :],
                                        e_g[:, u, lo:],
                                        start=(kc == 0), stop=(kc == nkc - 1),
                                        skip_group_check=True)
                                    nc.tensor.matmul(
                                        den_ps[:, lo:],
                                        ones_sb[:],
                                        e_g[:, u, lo:],
                                        start=(kc == 0), stop=(kc == nkc - 1),
                                        skip_group_check=True)
                            rcp = rcp_pool.tile([P, QT], F32, tag="rcp")
                            nc.vector.reciprocal_approx_fast(rcp[:], den_ps[:])
                            nc.vector.tensor_mul(
                                ctx_t[:, h, qt * QT:(qt + 1) * QT],
                                ctx_ps[:], rcp[:])

                    for b in range(B):
                        if b == 1:
                            # wo arrives during attention, ready for phase C
                            nc.sync.dma_start(wo_sb[:], woT_v.bitcast(F32R))
                        ctx_t = ctx_pool.tile([P, HPC, T], F32R, tag="ctx")
                        ctx_ts.append(ctx_t)
                        for h in range(HPC):
                            do_pair(b, h, ctx_t)

                # ---------------- Phase C: output projection ----------------
                with tc.tile_pool(name="pp_o", bufs=4, space="PSUM") as pp_o:
                    ncopy = 0
                    for b in range(B):
                        ctx_t = ctx_ts[b]
                        for tb2 in range(TPB):
                            t0 = b * T + tb2 * P
                            ost = ost_pool.tile([P, D], BF16, tag="ost")
                            for od in range(NOD):
                                ods = slice(od * QT, (od + 1) * QT)
                                ps0 = pp_o.tile([P, QT], F32, tag="po")
                                nc.tensor.matmul(
                                    ps0[:], ctx_t[:, 0, tb2 * P:(tb2 + 1) * P],
                                    wo_sb[:, 0, ods], start=True, stop=False)
                                nc.tensor.matmul(
                                    ps0[:], ctx_t[:, 1, tb2 * P:(tb2 + 1) * P],
                                    wo_sb[:, 1, ods], start=False, stop=True)
                                if ncopy % 2 == 0:
                                    nc.vector.tensor_copy(ost[:, ods], ps0[:])
                                else:
                                    nc.scalar.copy(ost[:, ods], ps0[:])
                                ncopy += 1
                            nc.sync.dma_start(out_p[t0:t0 + P, :], ost[:])

    nc.compile()
    return nc


def _get_module():
    if "nc" not in _CACHE:
        _CACHE["nc"] = _build_module()
    return _CACHE["nc"]


def _make_tri():
    return (np.arange(KC)[:, None] <= np.arange(KC)[None, :]).astype(np.float32)


def _run(x, Wq, Wk, Wv, Wo, bo, trace=False):
    from concourse import bass_utils

    nc = _get_module()
    x = np.asarray(x, dtype=np.float32)
    xT = np.ascontiguousarray(x.reshape(NTOK, D).T)
    tri = _make_tri()
    Wq = np.asarray(Wq, np.float32)
    Wk = np.asarray(Wk, np.float32)
    Wv = np.asarray(Wv, np.float32)
    Wo = np.asarray(Wo, np.float32)
    in_maps = []
    for c in range(N_CORES):
        sl = slice(c * DPC, (c + 1) * DPC)
        in_maps.append({
            "xT": xT,
            "wqT": np.ascontiguousarray(Wq[sl, :].T),
            "wkT": np.ascontiguousarray(Wk[sl, :].T),
            "wvT": np.ascontiguousarray(Wv[sl, :].T),
            "woT": np.ascontiguousarray(Wo[:, sl].T),
            "tri": tri,
        })
    res = bass_utils.run_bass_kernel_spmd(
        nc, in_maps, core_ids=list(range(N_CORES)), trace=trace)

    out = np.zeros((NTOK, D), np.float32)
    k = np.empty((NTOK, D), np.float32)
    v = np.empty((NTOK, D), np.float32)
    for c, r in enumerate(res.results):
        sl = slice(c * DPC, (c + 1) * DPC)
        out += np.asarray(r["out_p"]).astype(np.float32)
        k[:, sl] = np.asarray(r["kT_out"]).T
        v[:, sl] = np.asarray(r["v_out"])
    out += np.asarray(bo, np.float32)[None, :]
    outs = (out.reshape(B, T, D), k.reshape(B, T, D), v.reshape(B, T, D))
    return outs, res


def kernel(x, Wq, Wk, Wv, Wo, bo):
    outs, _ = _run(x, Wq, Wk, Wv, Wo, bo, trace=False)
    return outs
